# revision 1
# baseline (speedup 1.0000x reference)
"""Policy-loss kernel for Trainium2, data-parallel across 8 NeuronCores.

Reference computation (B=16384, m=2048, action has 4*m columns):
    seg_max = max(action.reshape(B, m, 4), axis=-1)        # [B, m]
    a_n     = mean(seg_max, axis=-1)                       # [B]
    v       = log(a_n) * a_n                               # [B]
    loss    = | mean(v * reward) + BETA * mean(v) |        # scalar

Sharding: rows (batch) split evenly over 8 cores (2048 rows each). Each core
streams its 2048x8192 f32 slice through SBUF in 16 tiles of [128, 8192],
computes per-row v with two strided tensor_tensor maxes + one
tensor_tensor_reduce (final max fused with the segment mean) on DVE and the
log on ACT, and returns per-partition partial sums [128, 2] =
(sum v*r, sum v). The host reduces the 8x128x2 partials and applies abs.
"""

import numpy as np

import concourse.bass as bass
import concourse.mybir as mybir
import concourse.tile as tile
from concourse.bass_utils import run_bass_kernel_spmd

BETA = 0.1
N_CORES = 8


def _sem_clear_compat(self, sem):
    """Replacement for BassGpSimd.sem_clear: the EVENT_SEMAPHORE_RANGE_CLEAR
    ISA op (opcode 176) fails this neuronxcc's codegen with "ISA wrong
    length". Emit one EventSemaphore sem-wr-imm 0 per semaphore instead —
    same architectural effect (zero the sems), encodes fine."""
    nums = list(sem) if isinstance(sem, range) else [sem.num]
    inst = None
    for n in nums:
        inst = self.add_instruction(
            mybir.InstEventSemaphore(
                name=f"semclr{n}_{self.bass.next_id()}",
                engine=self.engine,
                ins=[],
                outs=[],
                sync_info=mybir.SyncInfo(
                    on_wait=[],
                    on_update=[
                        mybir.SyncUpdate(
                            sync_type="semaphore",
                            id=n,
                            update_mode="sem-wr-imm",
                            update_value=0,
                        )
                    ],
                ),
            )
        )
    return inst


bass.BassGpSimd.sem_clear = _sem_clear_compat
B = 16384
COLS = 8192          # 4 * mobile_num
M = COLS // 4        # 2048 segments per row
ROWS_PER_CORE = B // N_CORES      # 2048
P = 128                           # SBUF partitions
NT = ROWS_PER_CORE // P           # 16 tiles per core

F32 = mybir.dt.float32


def _build_nc(rows_per_core: int = ROWS_PER_CORE, cols: int = COLS) -> bass.Bass:
    """Raw-bass pipeline (this neuronxcc rejects Tile's multi-wait DMAs):
    SP streams action tiles into a double buffer, DVE does the pairwise max
    tree, ACT does mean+log+v. Manual semaphores, waits are standalone
    sequencer instructions."""
    nt = rows_per_core // P
    m = cols // 4
    Ln = mybir.ActivationFunctionType.Ln
    Copy = mybir.ActivationFunctionType.Copy
    MAX = mybir.AluOpType.max

    nc = bass.Bass()
    a_ext = nc.declare_dram_parameter("action", [rows_per_core, cols], F32, isOutput=False)
    r_ext = nc.declare_dram_parameter("rt", [P, nt], F32, isOutput=False)
    out_ext = nc.declare_dram_parameter("partial", [P, 2], F32, isOutput=True)

    from contextlib import ExitStack

    with ExitStack() as stack:
        at0 = stack.enter_context(nc.sbuf_tensor([P, cols], F32))
        at1 = stack.enter_context(nc.sbuf_tensor([P, cols], F32))
        m1b = stack.enter_context(nc.sbuf_tensor([P, 2 * m], F32))
        seg0 = stack.enter_context(nc.sbuf_tensor([P, m], F32))
        seg1 = stack.enter_context(nc.sbuf_tensor([P, m], F32))
        sg2 = stack.enter_context(nc.sbuf_tensor([P, m], F32))
        a_all = stack.enter_context(nc.sbuf_tensor([P, nt], F32))
        v_all = stack.enter_context(nc.sbuf_tensor([P, nt], F32))
        rt = stack.enter_context(nc.sbuf_tensor([P, nt], F32))
        vr = stack.enter_context(nc.sbuf_tensor([P, nt], F32))
        lg = stack.enter_context(nc.sbuf_tensor([P, 1], F32))
        s1 = stack.enter_context(nc.sbuf_tensor([P, 1], F32))
        s2 = stack.enter_context(nc.sbuf_tensor([P, 1], F32))
        outt = stack.enter_context(nc.sbuf_tensor([P, 2], F32))
        dma_s0 = stack.enter_context(nc.semaphore("dma_s0"))
        dma_s1 = stack.enter_context(nc.semaphore("dma_s1"))
        rt_sem = stack.enter_context(nc.semaphore("rt_sem"))
        out_sem = stack.enter_context(nc.semaphore("out_sem"))
        dve_free = stack.enter_context(nc.semaphore("dve_free"))
        dve_seg = stack.enter_context(nc.semaphore("dve_seg"))
        act_done = stack.enter_context(nc.semaphore("act_done"))
        act_chain = stack.enter_context(nc.semaphore("act_chain"))
        block = stack.enter_context(nc.Block())
        ats = [at0, at1]
        segs = [seg0, seg1]
        dma_s = [dma_s0, dma_s1]

        @block.sync
        def _(sync):
            sync.dma_start(out=rt[:], in_=r_ext[:]).then_inc(rt_sem, 16)
            for i in range(nt):
                if i >= 2:
                    # at[i%2] WAR: max1 of tile i-2 consumed it
                    sync.wait_ge(dve_free, i - 1)
                    # trivially-true direct wait so the slot-sem inc is ordered
                    sync.wait_ge(dma_s[i % 2], 16 * (i // 2))
                sync.dma_start(
                    out=ats[i % 2][:], in_=a_ext[bass.ts(i, P), :]
                ).then_inc(dma_s[i % 2], 16)
            sync.wait_ge(act_done, nt + 2)
            sync.dma_start(out=out_ext[:], in_=outt[:]).then_inc(out_sem, 16)
            sync.wait_ge(out_sem, 16)

        @block.vector
        def _(vector):
            for i in range(nt):
                at = ats[i % 2]
                seg = segs[i % 2]
                vector.wait_ge(dma_s[i % 2], 16 * (i // 2 + 1))
                if i >= 1:
                    # m1b WAR: max2 of tile i-1 read it
                    vector.wait_ge(dve_seg, i)
                vector.tensor_tensor(
                    out=m1b[:], in0=at[:, 0::2], in1=at[:, 1::2], op=MAX
                ).then_inc(dve_free, 1)
                # m1b RAW (same engine, needs explicit sem for ordering model)
                vector.wait_ge(dve_free, i + 1)
                if i >= 2:
                    # seg[i%2] WAR: ACT reduce of tile i-2 read it
                    vector.wait_ge(act_chain, 2 * i - 3)
                vector.tensor_tensor(
                    out=seg[:], in0=m1b[:, 0::2], in1=m1b[:, 1::2], op=MAX
                ).then_inc(dve_seg, 1)
            # final partial sums over the nt per-tile v values
            vector.wait_ge(act_done, nt)
            vector.wait_ge(rt_sem, 16)
            vector.tensor_tensor(
                out=vr[:], in0=v_all[:], in1=rt[:], op=mybir.AluOpType.mult
            ).then_inc(dve_free, 1)
            vector.wait_ge(dve_free, nt + 1)
            vector.reduce_sum(
                out=s1[:], in_=vr[:], axis=mybir.AxisListType.X
            ).then_inc(dve_seg, 1)
            vector.reduce_sum(
                out=s2[:], in_=v_all[:], axis=mybir.AxisListType.X
            ).then_inc(dve_seg, 1)

        @block.scalar
        def _(scalar):
            for i in range(nt):
                seg = segs[i % 2]
                a_n = a_all[:, i : i + 1]
                scalar.wait_ge(dve_seg, i + 1)
                if i >= 1:
                    # sg2 WAW vs reduce of tile i-1
                    scalar.wait_ge(act_chain, 2 * i - 1)
                # out = seg * (1/m); accum_out = mean(seg) = a_n
                scalar.activation(
                    out=sg2[:], in_=seg[:], func=Copy, bias=0.0, scale=1.0 / m,
                    accum_out=a_n,
                ).then_inc(act_chain, 1)
                scalar.wait_ge(act_chain, 2 * i + 1)
                if i >= 1:
                    # lg WAR: v-write of tile i-1 read it
                    scalar.wait_ge(act_done, i)
                scalar.activation(out=lg[:], in_=a_n, func=Ln).then_inc(act_chain, 1)
                scalar.wait_ge(act_chain, 2 * i + 2)
                # v = log(a_n) * a_n into column i of v_all
                scalar.activation(
                    out=v_all[:, i : i + 1], in_=lg[:], func=Copy, bias=0.0,
                    scale=a_n,
                ).then_inc(act_done, 1)
            scalar.wait_ge(dve_seg, nt + 1)
            scalar.copy(out=outt[:, 0:1], in_=s1[:]).then_inc(act_done, 1)
            scalar.wait_ge(dve_seg, nt + 2)
            scalar.copy(out=outt[:, 1:2], in_=s2[:]).then_inc(act_done, 1)

    return nc


def _make_in_maps(reward: np.ndarray, action: np.ndarray, n_cores: int = N_CORES):
    rows_per_core = action.shape[0] // n_cores
    nt = rows_per_core // P
    a_sh = np.ascontiguousarray(action, dtype=np.float32).reshape(
        n_cores, rows_per_core, action.shape[1]
    )
    # rt[c][p, i] = reward[c*rows_per_core + i*P + p]
    r_sh = np.ascontiguousarray(reward, dtype=np.float32).reshape(
        n_cores, nt, P
    ).transpose(0, 2, 1)
    return [
        {"action": a_sh[c], "rt": np.ascontiguousarray(r_sh[c])}
        for c in range(n_cores)
    ]


def _run(q_eval, reward, action, trace: bool = False):
    nc = _build_nc()
    in_maps = _make_in_maps(np.asarray(reward), np.asarray(action))
    res = run_bass_kernel_spmd(nc, in_maps, list(range(N_CORES)), trace=trace)
    partials = np.stack([res.results[c]["partial"] for c in range(N_CORES)])
    s1 = float(partials[:, :, 0].sum(dtype=np.float64))
    s2 = float(partials[:, :, 1].sum(dtype=np.float64))
    loss = np.float32(abs(np.float32(s1 / B) + np.float32(BETA) * np.float32(s2 / B)))
    return np.asarray(loss, dtype=np.float32), res


def kernel(q_eval, reward, action):
    out, _ = _run(q_eval, reward, action)
    return out



# revision 5
# speedup vs baseline: 1.0483x; 1.0483x over previous
"""Policy-loss kernel for Trainium2, data-parallel across 8 NeuronCores.

Reference computation (B=16384, m=2048, action has 4*m columns):
    seg_max = max(action.reshape(B, m, 4), axis=-1)        # [B, m]
    a_n     = mean(seg_max, axis=-1)                       # [B]
    v       = log(a_n) * a_n                               # [B]
    loss    = | mean(v * reward) + BETA * mean(v) |        # scalar

Sharding: rows (batch) split evenly over 8 cores (2048 rows each). Each core
streams its 2048x8192 f32 slice through SBUF: 15 tiles of [128, 8192] plus the
last tile in 4 column chunks of [128, 2048] so the post-stream compute tail is
short. Four action buffers keep the DMA ring stocked ~2 tiles ahead of the
DVE consumer, so the 16 SDMA engines run back-to-back at the HBM limit instead
of idling on the completion-sem -> max1 -> issue chain (the 2-buffer version
spent ~13% of the stream window with the DMA ring dry). Per tile DVE does the
pairwise max tree, ACT does mean (Copy with accum_out) + ln + v, and DVE
reduces v and v*r directly into the [128, 2] output tile. The host reduces the
8x128x2 partials and applies abs.

(tensor_tensor_reduce would fuse max2 with the segment mean, but this
neuronxcc's codegen rejects it with "ISA wrong length" — same class of failure
as EVENT_SEMAPHORE_RANGE_CLEAR below.)
"""

import numpy as np

import concourse.bass as bass
import concourse.mybir as mybir
import concourse.tile as tile
from concourse.bass_utils import run_bass_kernel_spmd

BETA = 0.1
N_CORES = 8


def _sem_clear_compat(self, sem):
    """Replacement for BassGpSimd.sem_clear: the EVENT_SEMAPHORE_RANGE_CLEAR
    ISA op (opcode 176) fails this neuronxcc's codegen with "ISA wrong
    length". Emit one EventSemaphore sem-wr-imm 0 per semaphore instead —
    same architectural effect (zero the sems), encodes fine."""
    nums = list(sem) if isinstance(sem, range) else [sem.num]
    inst = None
    for n in nums:
        inst = self.add_instruction(
            mybir.InstEventSemaphore(
                name=f"semclr{n}_{self.bass.next_id()}",
                engine=self.engine,
                ins=[],
                outs=[],
                sync_info=mybir.SyncInfo(
                    on_wait=[],
                    on_update=[
                        mybir.SyncUpdate(
                            sync_type="semaphore",
                            id=n,
                            update_mode="sem-wr-imm",
                            update_value=0,
                        )
                    ],
                ),
            )
        )
    return inst


bass.BassGpSimd.sem_clear = _sem_clear_compat
B = 16384
COLS = 8192          # 4 * mobile_num
M = COLS // 4        # 2048 segments per row
ROWS_PER_CORE = B // N_CORES      # 2048
P = 128                           # SBUF partitions
NT = ROWS_PER_CORE // P           # 16 tiles per core
NFULL = NT - 1                    # full-width tiles; the last is chunked
NCH = 4                           # column chunks of the last tile
CHC = COLS // NCH                 # 2048 cols per chunk
SEGC = CHC // 4                   # 512 segments per chunk
NBUF = 4                          # action buffer ring depth

F32 = mybir.dt.float32


def _build_nc(rows_per_core: int = ROWS_PER_CORE, cols: int = COLS) -> bass.Bass:
    """Raw-bass pipeline (this neuronxcc rejects Tile's multi-wait DMAs):
    SP streams action tiles into a 4-deep buffer ring, DVE does the pairwise
    max tree, ACT does mean+log+v. Manual semaphores, waits are standalone
    sequencer instructions; one DMA-completion sem per buffer slot (baseline
    idiom) so each sem's increments stay totally ordered."""
    m = cols // 4
    Ln = mybir.ActivationFunctionType.Ln
    Copy = mybir.ActivationFunctionType.Copy
    MAX = mybir.AluOpType.max

    nc = bass.Bass()
    a_ext = nc.declare_dram_parameter("action", [rows_per_core, cols], F32, isOutput=False)
    r_ext = nc.declare_dram_parameter("rt", [P, NT], F32, isOutput=False)
    out_ext = nc.declare_dram_parameter("partial", [P, 2], F32, isOutput=True)

    from contextlib import ExitStack

    with ExitStack() as stack:
        ats = [
            stack.enter_context(nc.sbuf_tensor(f"at{k}", [P, cols], F32))
            for k in range(NBUF)
        ]
        m1b = stack.enter_context(nc.sbuf_tensor([P, cols // 2], F32))
        seg0 = stack.enter_context(nc.sbuf_tensor([P, m], F32))
        seg1 = stack.enter_context(nc.sbuf_tensor([P, m], F32))
        sg2 = stack.enter_context(nc.sbuf_tensor([P, m], F32))
        a_all = stack.enter_context(nc.sbuf_tensor([P, NT], F32))
        a_nc = stack.enter_context(nc.sbuf_tensor([P, NCH], F32))
        sc4 = stack.enter_context(nc.sbuf_tensor([P, NCH], F32))
        v_all = stack.enter_context(nc.sbuf_tensor([P, NT], F32))
        rt = stack.enter_context(nc.sbuf_tensor([P, NT], F32))
        vr = stack.enter_context(nc.sbuf_tensor([P, NT], F32))
        lg = stack.enter_context(nc.sbuf_tensor([P, 1], F32))
        outt = stack.enter_context(nc.sbuf_tensor([P, 2], F32))
        dma_s = [
            stack.enter_context(nc.semaphore(f"dma_s{k}")) for k in range(NBUF)
        ]
        dma_c = [
            stack.enter_context(nc.semaphore(f"dma_c{k}")) for k in range(NCH)
        ]
        rt_sem = stack.enter_context(nc.semaphore("rt_sem"))
        out_sem = stack.enter_context(nc.semaphore("out_sem"))
        s_max1 = stack.enter_context(nc.semaphore("s_max1"))
        s_max2 = stack.enter_context(nc.semaphore("s_max2"))
        s_mean = stack.enter_context(nc.semaphore("s_mean"))
        s_act = stack.enter_context(nc.semaphore("s_act"))
        s_v = stack.enter_context(nc.semaphore("s_v"))
        s_fin = stack.enter_context(nc.semaphore("s_fin"))
        block = stack.enter_context(nc.Block())
        segs = [seg0, seg1]

        @block.sync
        def _(sync):
            for i in range(NFULL):
                if i >= NBUF:
                    # at[i%NBUF] WAR: max1 of tile i-NBUF consumed it
                    sync.wait_ge(s_max1, i - NBUF + 1)
                    # trivially-true direct wait so the slot-sem inc is ordered
                    sync.wait_ge(dma_s[i % NBUF], 16 * (i // NBUF))
                sync.dma_start(
                    out=ats[i % NBUF][:], in_=a_ext[bass.ts(i, P), :]
                ).then_inc(dma_s[i % NBUF], 16)
                if i == NBUF - 1:
                    sync.dma_start(out=rt[:], in_=r_ext[:]).then_inc(rt_sem, 16)
            # last tile in NCH column chunks into ats[NBUF-1]; its previous
            # user is tile NFULL-NBUF, so the max1 count must reach NFULL-NBUF+1
            sync.wait_ge(s_max1, NFULL - NBUF + 1)
            sync.wait_ge(dma_s[NBUF - 1], 16 * (NFULL // NBUF))
            for c in range(NCH):
                sync.dma_start(
                    out=ats[NBUF - 1][:, c * CHC : (c + 1) * CHC],
                    in_=a_ext[bass.ts(NFULL, P), c * CHC : (c + 1) * CHC],
                ).then_inc(dma_c[c], 16)
            sync.wait_ge(s_fin, 3)
            sync.dma_start(out=out_ext[:], in_=outt[:]).then_inc(out_sem, 16)
            sync.wait_ge(out_sem, 16)

        @block.vector
        def _(vector):
            for i in range(NFULL):
                at = ats[i % NBUF]
                vector.wait_ge(dma_s[i % NBUF], 16 * (i // NBUF + 1))
                if i >= 1:
                    # m1b WAR: max2 of tile i-1 read it
                    vector.wait_ge(s_max2, i)
                vector.tensor_tensor(
                    out=m1b[:], in0=at[:, 0::2], in1=at[:, 1::2], op=MAX
                ).then_inc(s_max1, 1)
                # m1b RAW (same engine, explicit sem for the ordering model)
                vector.wait_ge(s_max1, i + 1)
                if i >= 2:
                    # seg[i%2] WAR: ACT mean of tile i-2 read it
                    vector.wait_ge(s_mean, i - 1)
                vector.tensor_tensor(
                    out=segs[i % 2][:], in0=m1b[:, 0::2], in1=m1b[:, 1::2], op=MAX
                ).then_inc(s_max2, 1)
            for c in range(NCH):
                at = ats[NBUF - 1]
                c0 = c * CHC
                vector.wait_ge(dma_c[c], 16)
                # m1b WAR: previous max2 read it
                vector.wait_ge(s_max2, NFULL + c)
                vector.tensor_tensor(
                    out=m1b[:, 0 : CHC // 2],
                    in0=at[:, c0 : c0 + CHC : 2], in1=at[:, c0 + 1 : c0 + CHC : 2],
                    op=MAX,
                ).then_inc(s_max1, 1)
                vector.wait_ge(s_max1, NFULL + c + 1)
                if c == 0:
                    # seg1 WAR: ACT mean of tile NFULL-2 (odd) read it
                    vector.wait_ge(s_mean, NFULL - 1)
                vector.tensor_tensor(
                    out=seg1[:, c * SEGC : (c + 1) * SEGC],
                    in0=m1b[:, 0 : CHC // 2 : 2], in1=m1b[:, 1 : CHC // 2 : 2],
                    op=MAX,
                ).then_inc(s_max2, 1)
            # final partial sums over the NT per-tile v values
            vector.wait_ge(s_v, NT)
            vector.wait_ge(rt_sem, 16)
            vector.tensor_tensor(
                out=vr[:], in0=v_all[:], in1=rt[:], op=mybir.AluOpType.mult
            ).then_inc(s_fin, 1)
            vector.wait_ge(s_fin, 1)
            vector.reduce_sum(
                out=outt[:, 0:1], in_=vr[:], axis=mybir.AxisListType.X
            ).then_inc(s_fin, 1)
            vector.wait_ge(s_fin, 2)
            vector.reduce_sum(
                out=outt[:, 1:2], in_=v_all[:], axis=mybir.AxisListType.X
            ).then_inc(s_fin, 1)

        @block.scalar
        def _(scalar):
            for i in range(NFULL):
                seg = segs[i % 2]
                a_n = a_all[:, i : i + 1]
                scalar.wait_ge(s_max2, i + 1)
                if i >= 1:
                    # sg2 WAW vs mean of tile i-1 (same engine, ordering model)
                    scalar.wait_ge(s_mean, i)
                # out = seg * (1/m); accum_out = mean(seg) = a_n
                scalar.activation(
                    out=sg2[:], in_=seg[:], func=Copy, bias=0.0, scale=1.0 / m,
                    accum_out=a_n,
                ).then_inc(s_mean, 1)
                scalar.wait_ge(s_mean, i + 1)
                if i >= 1:
                    # lg WAR: v-write of tile i-1 read it
                    scalar.wait_ge(s_v, i)
                scalar.activation(out=lg[:], in_=a_n, func=Ln).then_inc(s_act, 1)
                scalar.wait_ge(s_act, i + 1)
                # v = log(a_n) * a_n into column i of v_all
                scalar.activation(
                    out=v_all[:, i : i + 1], in_=lg[:], func=Copy, bias=0.0,
                    scale=a_n,
                ).then_inc(s_v, 1)
            # chunked last tile: per-chunk partial means, then combine
            for c in range(NCH):
                scalar.wait_ge(s_max2, NFULL + c + 1)
                scalar.wait_ge(s_mean, NFULL + c)
                scalar.activation(
                    out=sg2[:, 0:SEGC], in_=seg1[:, c * SEGC : (c + 1) * SEGC],
                    func=Copy, bias=0.0, scale=1.0 / m,
                    accum_out=a_nc[:, c : c + 1],
                ).then_inc(s_mean, 1)
            a_n = a_all[:, NFULL : NFULL + 1]
            scalar.wait_ge(s_mean, NFULL + NCH)
            scalar.activation(
                out=sc4[:], in_=a_nc[:], func=Copy, bias=0.0, scale=1.0,
                accum_out=a_n,
            ).then_inc(s_mean, 1)
            scalar.wait_ge(s_mean, NFULL + NCH + 1)
            scalar.wait_ge(s_v, NFULL)
            scalar.activation(out=lg[:], in_=a_n, func=Ln).then_inc(s_act, 1)
            scalar.wait_ge(s_act, NT)
            scalar.activation(
                out=v_all[:, NFULL:NT], in_=lg[:], func=Copy, bias=0.0,
                scale=a_n,
            ).then_inc(s_v, 1)

    return nc


def _make_in_maps(reward: np.ndarray, action: np.ndarray, n_cores: int = N_CORES):
    rows_per_core = action.shape[0] // n_cores
    nt = rows_per_core // P
    a_sh = np.ascontiguousarray(action, dtype=np.float32).reshape(
        n_cores, rows_per_core, action.shape[1]
    )
    # rt[c][p, i] = reward[c*rows_per_core + i*P + p]
    r_sh = np.ascontiguousarray(reward, dtype=np.float32).reshape(
        n_cores, nt, P
    ).transpose(0, 2, 1)
    return [
        {"action": a_sh[c], "rt": np.ascontiguousarray(r_sh[c])}
        for c in range(n_cores)
    ]


def _run(q_eval, reward, action, trace: bool = False):
    nc = _build_nc()
    in_maps = _make_in_maps(np.asarray(reward), np.asarray(action))
    res = run_bass_kernel_spmd(nc, in_maps, list(range(N_CORES)), trace=trace)
    partials = np.stack([res.results[c]["partial"] for c in range(N_CORES)])
    s1 = float(partials[:, :, 0].sum(dtype=np.float64))
    s2 = float(partials[:, :, 1].sum(dtype=np.float64))
    loss = np.float32(abs(np.float32(s1 / B) + np.float32(BETA) * np.float32(s2 / B)))
    return np.asarray(loss, dtype=np.float32), res


def kernel(q_eval, reward, action):
    out, _ = _run(q_eval, reward, action)
    return out
